# revision 6
# baseline (speedup 1.0000x reference)
"""Trainium2 Bass kernel for the ContextComputer GNN message-passing module.

Computation (per batch row b):
    W1, W2 = W[:D], W[D:]
    mjm_j  = memory_j * mask_j                       # [N, D]
    a_i    = memory_i @ W1                           # [N, D]
    c_j    = mjm_j @ W2                              # [N, D]
    ctx_i  = sum_{j != i} sigmoid(a_i + c_j + bias) * mjm_j

Sharding: pure data parallel over batch B across the 8 NeuronCores
(B=8192 -> 1024 rows per core); W/b replicated.

Per-core kernel layout ("layout B"): batch rows on the 128 SBUF
partitions, feature dim (D=512) on the free axis.
  - PE matmuls need the contraction dim (d) on partitions, so an m^T
    copy is produced by casting memory to bf16 into a DRAM scratch and
    using the hardware transpose-DMA (xbar path is bf16-only).
  - a'_i = m_i @ W1 + 1*bias is accumulated in PSUM (bias via a rank-1
    ones matmul). c_j = mask_j * (m_j @ W2) applies the mask as a
    per-partition scale in the ScalarE PSUM->SBUF copy.
  - Pairwise stage is batched over all 6 j per i with wide DVE/ACT
    instructions; the off-diagonal j-sum is 4 chained adds, the last
    writing fp32 for the output store.
"""

import numpy as np

import concourse.bass as bass
import concourse.mybir as mybir
import concourse.tile as tile
from concourse.bass_utils import run_bass_kernel_spmd

B, N, D = 8192, 6, 512
P = 128
DC = D // P  # 4 contraction chunks of 128
NCORES = 8
BLOC = B // NCORES

F32 = mybir.dt.float32
BF16 = mybir.dt.bfloat16

_ADD = mybir.AluOpType.add
_MULT = mybir.AluOpType.mult
_SIGMOID = mybir.ActivationFunctionType.Sigmoid

_nc_cache = {}


def _split_excess_waits(nc, max_waits=1):
    """The pinned walrus build only supports one sync-wait slot per
    instruction; hoist extra Tile-emitted waits onto standalone
    same-engine EventSemaphore instructions (NX dispatcher-level waits,
    so ordering semantics are preserved)."""
    f = nc.m.functions[0]
    for blk in f.blocks:
        new = []
        for ins in blk.instructions:
            si = getattr(ins, "sync_info", None)
            eng = getattr(ins, "engine", None)
            if si is not None and si.on_wait and len(si.on_wait) > max_waits and eng is not None:
                waits = list(si.on_wait)
                extra, keep = waits[:-max_waits], waits[-max_waits:]
                for k, w in enumerate(extra):
                    new.append(
                        mybir.InstEventSemaphore(
                            name=f"{ins.name}_xw{k}",
                            opcode="EventSemaphore",
                            engine=eng,
                            ins=[],
                            outs=[],
                            sync_info=mybir.SyncInfo(on_wait=[w], on_update=[]),
                        )
                    )
                si.on_wait = keep
            new.append(ins)
        blk.instructions[:] = new


def build(bloc=BLOC):
    nbt = bloc // P
    nc = bass.Bass()
    mem = nc.declare_dram_parameter("memory", [bloc, N, D], F32, isOutput=False)
    msk = nc.declare_dram_parameter("mask", [bloc, N, 1], F32, isOutput=False)
    w_p = nc.declare_dram_parameter("W", [2 * D, D], F32, isOutput=False)
    b_p = nc.declare_dram_parameter("b", [D], F32, isOutput=False)
    out = nc.declare_dram_parameter("context", [bloc, N, D], F32, isOutput=True)
    mbf = nc.dram_tensor("mbf", [bloc, N, D], BF16)

    with tile.TileContext(nc) as tc:
        with (
            tc.tile_pool(name="const", bufs=1) as constp,
            tc.tile_pool(name="mt", bufs=2) as mtp,
            tc.tile_pool(name="work", bufs=2) as work,
            tc.tile_pool(name="pair", bufs=2) as pairp,
            tc.tile_pool(name="acc", bufs=4) as accp,
            tc.tile_pool(name="outp", bufs=8) as outp,
            tc.tile_pool(name="psum", bufs=3, space="PSUM") as psp,
        ):
            # ---- constants: W (cast to bf16), bias, ones row ----
            wt = {}
            for h in range(2):  # 0 -> W1, 1 -> W2
                for dc in range(DC):
                    t = constp.tile([P, D], BF16, tag=f"w{h}{dc}")
                    nc.gpsimd.dma_start(
                        out=t[:], in_=w_p[h * D + dc * P : h * D + (dc + 1) * P, :]
                    )
                    wt[h, dc] = t
            bias_t = constp.tile([1, D], BF16, tag="bias")
            nc.gpsimd.dma_start(out=bias_t[:], in_=b_p[None, :])
            ones_t = constp.tile([1, P], BF16, tag="ones")
            nc.vector.memset(ones_t[:], 1.0)

            for bt in range(nbt):
                bsl = slice(bt * P, (bt + 1) * P)

                mask_t = work.tile([P, N], F32, tag="mask")
                nc.sync.dma_start(out=mask_t[:], in_=msk[bsl, :, 0])

                # natural-layout memory, cast to bf16 on load (single DMA so
                # downstream consumers wait on one queue sem)
                m_all = work.tile([P, N * D], BF16, tag="mnat")
                nc.gpsimd.dma_start(
                    out=m_all.rearrange("p (n d) -> p n d", n=N), in_=mem[bsl]
                )
                # u = mask_j * m_j  (per-partition scalar -> 4x mode)
                mask_bf = work.tile([P, N], BF16, tag="maskbf")
                nc.gpsimd.tensor_copy(out=mask_bf[:], in_=mask_t[:])
                u_all = work.tile([P, N * D], BF16, tag="u")
                for j in range(N):
                    nc.gpsimd.tensor_tensor(
                        out=u_all[:, j * D : (j + 1) * D],
                        in0=m_all[:, j * D : (j + 1) * D],
                        in1=mask_bf[:, j : j + 1].broadcast_to([P, D]),
                        op=_MULT,
                    )
                # bf16 scratch copy of memory (source for transpose-DMA)
                nc.sync.dma_start(
                    out=mbf[bsl],
                    in_=m_all.rearrange("p (n d) -> p n d", n=N),
                )
                # m^T tiles for this b-tile: [128 d, 128 b] per (head-col j, dc)
                mt = {}
                for j in range(N):
                    for dc in range(DC):
                        t = mtp.tile([P, P], BF16, tag=f"mt{j}{dc}")
                        nc.sync.dma_start(
                            out=t[:],
                            in_=mbf[bsl, j, dc * P : (dc + 1) * P],
                            transpose=True,
                        )
                        mt[j, dc] = t

                # ---- matmuls ----
                a_all = work.tile([P, N * D], BF16, tag="a")
                c_all = work.tile([P, N * D], BF16, tag="c")
                for i in range(N):
                    a_ps = psp.tile([P, D], F32, tag="aps")
                    for dc in range(DC):
                        nc.tensor.matmul(
                            out=a_ps[:],
                            lhsT=mt[i, dc][:],
                            rhs=wt[0, dc][:],
                            start=(dc == 0),
                            stop=False,
                        )
                    nc.tensor.matmul(
                        out=a_ps[:],
                        lhsT=ones_t[:],
                        rhs=bias_t[:],
                        start=False,
                        stop=True,
                    )
                    nc.scalar.copy(out=a_all[:, i * D : (i + 1) * D], in_=a_ps[:])
                for j in range(N):
                    c_ps = psp.tile([P, D], F32, tag="cps")
                    for dc in range(DC):
                        nc.tensor.matmul(
                            out=c_ps[:],
                            lhsT=mt[j, dc][:],
                            rhs=wt[1, dc][:],
                            start=(dc == 0),
                            stop=(dc == DC - 1),
                        )
                    # c_j = mask_j * (m_j @ W2): scale in the PSUM->SBUF copy
                    nc.scalar.mul(
                        out=c_all[:, j * D : (j + 1) * D],
                        in_=c_ps[:],
                        mul=mask_t[:, j : j + 1],
                    )

                # ---- pairwise sigmoid gating ----
                c_v = c_all.rearrange("p (j f) -> p j f", j=N)
                u_v = u_all.rearrange("p (j f) -> p j f", j=N)
                for i in range(N):
                    a_b = (
                        a_all[:, i * D : (i + 1) * D]
                        .rearrange("p (j f) -> p j f", j=1)
                        .broadcast_to([P, N, D])
                    )
                    t_all = pairp.tile([P, N * D], BF16, tag="t")
                    nc.vector.tensor_tensor(
                        out=t_all.rearrange("p (j f) -> p j f", j=N),
                        in0=a_b,
                        in1=c_v,
                        op=_ADD,
                    )
                    g_all = pairp.tile([P, N * D], BF16, tag="g")
                    nc.scalar.activation(out=g_all[:], in_=t_all[:], func=_SIGMOID)
                    p_all = pairp.tile([P, N * D], BF16, tag="pp")
                    nc.vector.tensor_tensor(
                        out=p_all[:], in0=g_all[:], in1=u_all[:], op=_MULT
                    )
                    js = [j for j in range(N) if j != i]
                    s = accp.tile([P, D], BF16, tag="s")
                    nc.vector.tensor_add(
                        out=s[:],
                        in0=p_all[:, js[0] * D : (js[0] + 1) * D],
                        in1=p_all[:, js[1] * D : (js[1] + 1) * D],
                    )
                    for j in js[2:-1]:
                        nc.vector.tensor_add(
                            out=s[:], in0=s[:], in1=p_all[:, j * D : (j + 1) * D]
                        )
                    ctx_t = outp.tile([P, D], F32, tag="ctx")
                    nc.vector.tensor_add(
                        out=ctx_t[:],
                        in0=s[:],
                        in1=p_all[:, js[-1] * D : (js[-1] + 1) * D],
                    )
                    nc.sync.dma_start(out=out[bsl, i, :], in_=ctx_t[:])
    _split_excess_waits(nc)
    return nc


def get_nc(bloc=BLOC):
    if bloc not in _nc_cache:
        _nc_cache[bloc] = build(bloc)
    return _nc_cache[bloc]


last_results = None


def kernel(**inputs):
    global last_results
    memory = np.ascontiguousarray(inputs["memory"], dtype=np.float32)
    mask = np.ascontiguousarray(inputs["mask"], dtype=np.float32)
    W = np.ascontiguousarray(inputs["W"], dtype=np.float32)
    b = np.ascontiguousarray(inputs["b"], dtype=np.float32)

    nc = get_nc()
    in_maps = [
        {
            "memory": memory[c * BLOC : (c + 1) * BLOC],
            "mask": mask[c * BLOC : (c + 1) * BLOC],
            "W": W,
            "b": b,
        }
        for c in range(NCORES)
    ]
    res = run_bass_kernel_spmd(nc, in_maps, list(range(NCORES)))
    last_results = res
    out = np.concatenate(
        [res.results[c]["context"] for c in range(NCORES)], axis=0
    )
    return out.astype(np.float32, copy=False)
